# revision 24
# baseline (speedup 1.0000x reference)
"""Causal attention (B=8, N=4096 flattened 64x64, d=128) on 8 trn2 cores.

Sharding: data-parallel over batch -- core b gets batch element b.

Per-core algorithm, transposed orientation (S^T[k, q] tiles, O^T[c, q]):

  fp8 fast path (q-chunks 1..7, i.e. rows with >= 512 causal keys):
    - Q/K/V quantized to fp8e4m3 on host. QK^T and PV/den matmuls run in
      DoubleRow perf mode (0.5 PE cycles/output column): the d=128
      contraction is split as [64 partitions x 2 subtiles] for S, and
      k-tiles are processed in adjacent PAIRS ([128, 2, *]) for PV/den.
    - exp via the "Schraudolph byte" trick: the fp8e4m3 bit pattern of
      exp(s/sqrt(128)) is byte(s*C1 + C2) with C1 = 8*log2(e)/sqrt(128),
      C2 = 56 (= 7*8, the e4m3 exponent bias in 1/8-octave units).
      tensor_scalar/activation(Copy) compute y = s*C1 + C2 and the
      fp32->uint8 output conversion (round-nearest, SATURATING at 0/255 --
      verified on HW) yields E = exp(.) directly as fp8 bytes. Saturation
      at 0 zeroes masked/tiny entries; 255 needs a +17 sigma score (never).
      The same formula runs on ACT (activation Copy), DVE and Pool
      (tensor_scalar), statically load-balanced across the three.
    - causal diagonal tiles: scalar_tensor_tensor folds the triangle mask
      into the additive operand (C2 on allowed cells, -1000 on masked), so
      masking costs nothing extra. Diagonal E tiles live in dedicated
      buffers whose never-written prefix is zeroed once at init.
    - denominator: ones8^T @ E pair-matmuls accumulate in PSUM (no DVE
      reduction chain at all).

  bf16 head path (q-chunk 0): rows with < 512 keys have too few terms for
  fp8 weight noise to average out, so chunk 0 runs the baseline-style
  bf16 pipeline (true exp on ACT -> bf16 E, bf16 V). Costs ~3us.

  outputs per core: outT [128, 4096] fp16 (unnormalized O^T), den [1,4096]
  fp32; host computes (outT / den).T in fp32.
"""

import math

import ml_dtypes
import numpy as np

import concourse.bacc as bacc
import concourse.mybir as mybir
import concourse.tile as tile
from concourse.bass import ts, ds
from concourse.bass_utils import run_bass_kernel_spmd

P = 128
NSEQ = 4096
QCH = 512              # query positions per chunk
NCH = NSEQ // QCH      # 8 chunks
N_CORES = 8
SCALE = 1.0 / math.sqrt(128.0)
C1 = 8.0 / math.log(2.0) / math.sqrt(128.0)   # score -> byte slope
C2 = 40.0   # e4m3 exponent bias*8 (56) minus 16: scales all E by 2^-2
            # (softmax-invariant) so byte 120 (=inf in IEEE e4m3) needs a
            # +6.9 sigma score instead of +5.6 (which does occur)
NEGB = -100000.0                              # masked-cell additive bias

F32 = mybir.dt.float32
F16 = mybir.dt.float16
BF16 = mybir.dt.bfloat16
F8 = mybir.dt.float8e4
U8 = mybir.dt.uint8
DR = mybir.MatmulPerfMode.DoubleRow
MULT = mybir.AluOpType.mult
ADD = mybir.AluOpType.add
EXP = mybir.ActivationFunctionType.Exp
COPY = mybir.ActivationFunctionType.Copy

_nc_cache = []


def _build():
    nc = bacc.Bacc("TRN2", target_bir_lowering=False, debug=False,
                   num_devices=N_CORES)
    qT8 = nc.dram_tensor("qT8", [P, NSEQ], F8, kind="ExternalInput").ap()
    kT8 = nc.dram_tensor("kT8", [P, NSEQ], F8, kind="ExternalInput").ap()
    qT0 = nc.dram_tensor("qT0", [P, QCH], BF16, kind="ExternalInput").ap()
    kT0 = nc.dram_tensor("kT0", [P, QCH], BF16, kind="ExternalInput").ap()
    v8 = nc.dram_tensor("v8", [P, NSEQ], F8, kind="ExternalInput").ap()
    v0 = nc.dram_tensor("v0", [P, 4 * P], BF16, kind="ExternalInput").ap()
    outT = nc.dram_tensor("outT", [P, NSEQ], F16, kind="ExternalOutput").ap()
    den = nc.dram_tensor("den", [1, NSEQ], F32, kind="ExternalOutput").ap()

    # ---- static engine schedule for the exp work --------------------
    # virtual-time greedy balancing; costs in ns per whole instruction.
    # (Pool/gpsimd cannot touch PSUM, so exp runs on ACT+DVE only; Pool
    # handles SBUF-side byte masks for ACT-routed diagonal tiles.)
    vt = {"act": 0.0, "dve": 0.0, "pool": 0.0}
    ACT_R, ACT_O = 0.68, 217.0
    DVE_R, DVE_O = 1.19, 230.0
    POOL_MASK = 480.0

    def pick2(act_cost, dve_cost):
        if vt["act"] + act_cost <= vt["dve"] + dve_cost:
            vt["act"] += act_cost
            return "act"
        vt["dve"] += dve_cost
        return "dve"

    with tile.TileContext(nc) as tc:
        with (
            tc.tile_pool(name="const", bufs=1) as cpool,
            tc.tile_pool(name="epool", bufs=6) as epool,
            tc.tile_pool(name="spool", bufs=2) as spool,
            tc.tile_pool(name="ps_s", bufs=3, space="PSUM") as ps_pool,
            tc.tile_pool(name="ps_o", bufs=1, space="PSUM") as po_pool,
            tc.tile_pool(name="ps_d", bufs=1, space="PSUM") as pd_pool,
        ):
            # ---- constants ----
            ones_bf = cpool.tile([P, P], BF16)
            nc.gpsimd.memset(ones_bf, 1.0)
            ones8 = cpool.tile([P, 2, P], F8)
            nc.gpsimd.memset(ones8, 1.0)
            # maskbias [P, QCH] f32: C2 everywhere except the leading
            # [P, P] triangle block where masked (q' < k) cells get NEGB
            maskbias = cpool.tile([P, QCH], F32)
            nc.vector.memset(maskbias, C2)
            nc.gpsimd.affine_select(
                out=maskbias[:, :P], in_=maskbias[:, :P],
                compare_op=mybir.AluOpType.is_ge, fill=NEGB,
                base=0, pattern=[[1, P]], channel_multiplier=-1)
            # tri0 [P, P] bf16: 1 where q' >= k else 0 (chunk-0 masks)
            scratch = cpool.tile([P, P], F32)
            nc.gpsimd.memset(scratch, 1.0)
            nc.gpsimd.affine_select(
                out=scratch, in_=scratch,
                compare_op=mybir.AluOpType.is_ge, fill=0.0,
                base=0, pattern=[[1, P]], channel_multiplier=-1)
            tri0 = cpool.tile([P, P], BF16)
            nc.vector.tensor_copy(tri0, scratch)
            # tri_f8 [P, P] fp8: same triangle as 0.0/1.0 (fp multiply by
            # exactly 0/1 masks the fp8 E bytes losslessly)
            tri_f8 = cpool.tile([P, P], F8)
            nc.vector.tensor_copy(tri_f8, scratch)

            # prefetch the Exp activation table while DMAs run
            tl = cpool.tile([1, 1], F32)
            nc.vector.memset(tl, 0.0)
            nc.scalar.activation(tl, tl, EXP, scale=1.0)

            # PE warmup: spin the clock up during the input DMA wait;
            # chunk 0's first den matmul (start=True) clears this bank
            warm_db = pd_pool.tile([P, QCH], F32, tag="db", name="warm")
            for _ in range(26):
                nc.tensor.matmul(warm_db[:, ds(0, P)], ones_bf, ones_bf,
                                 start=True, stop=True)

            # ---- input DMAs ----
            # sync queue: chunk-0 bf16 head then the K fp8 stream;
            # scalar queue: V + Q fp8 stream; pieces in first-use order
            v0_sb = cpool.tile([P, 4, P], BF16)
            qT0_sb = cpool.tile([P, QCH], BF16)
            kT0_sb = cpool.tile([P, QCH], BF16)
            qT8_sb = cpool.tile([P, NSEQ], F8)
            kT8_sb = cpool.tile([P, NSEQ], F8)
            v8_sb = cpool.tile([P, NSEQ // P, P], F8)
            nc.sync.dma_start(kT0_sb, kT0)
            nc.sync.dma_start(kT8_sb[:, :2 * QCH], kT8[:, :2 * QCH])
            nc.sync.dma_start(kT8_sb[:, 2 * QCH:4 * QCH],
                              kT8[:, 2 * QCH:4 * QCH])
            nc.sync.dma_start(kT8_sb[:, 4 * QCH:], kT8[:, 4 * QCH:])
            nc.scalar.dma_start(qT0_sb, qT0)
            nc.scalar.dma_start(v0_sb, v0.rearrange("p (j c) -> p j c", c=P))
            nc.scalar.dma_start(qT8_sb[:, :2 * QCH], qT8[:, :2 * QCH])
            nc.scalar.dma_start(
                v8_sb[:, :8, :],
                v8[:, :8 * P].rearrange("p (j c) -> p j c", c=P))
            nc.scalar.dma_start(qT8_sb[:, 2 * QCH:4 * QCH],
                                qT8[:, 2 * QCH:4 * QCH])
            nc.scalar.dma_start(
                v8_sb[:, 8:, :],
                v8[:, 8 * P:].rearrange("p (j c) -> p j c", c=P))
            nc.scalar.dma_start(qT8_sb[:, 4 * QCH:], qT8[:, 4 * QCH:])

            # dedicated diagonal E buffers (never-written prefixes stay 0)
            ediagA = cpool.tile([P, 2, QCH], F8)   # dd = 0, 1
            ediagB = cpool.tile([P, 2, QCH], F8)   # dd = 2, 3
            nc.vector.memset(ediagA.bitcast(U8), 0)
            nc.vector.memset(ediagB.bitcast(U8), 0)
            e0 = cpool.tile([P, 4, QCH], BF16)     # chunk-0 E (bf16)
            nc.gpsimd.memset(e0, 0.0)

            den_all = cpool.tile([1, NSEQ], F32)

            def flush_chunk(t, o_ps, db_ps):
                o_sb = spool.tile([P, QCH], F16, tag="osb", name=f"osb{t}")
                if t == NCH - 1:
                    # tail: halve latency -- split the copy across ACT+DVE
                    # and DMA each half as soon as it lands
                    h = QCH // 2
                    nc.vector.tensor_copy(den_all[:, ts(t, QCH)],
                                          db_ps[0:1, :])
                    nc.sync.dma_start(den, den_all)
                    nc.scalar.copy(o_sb[:, :h], o_ps[:, :h])
                    nc.sync.dma_start(outT[:, ds(t * QCH, h)], o_sb[:, :h])
                    nc.vector.tensor_copy(o_sb[:, h:], o_ps[:, h:])
                    nc.sync.dma_start(outT[:, ds(t * QCH + h, h)],
                                      o_sb[:, h:])
                    return
                eng = pick2(QCH * ACT_R + ACT_O, QCH * DVE_R + DVE_O)
                if eng == "act":
                    nc.scalar.copy(o_sb, o_ps)
                else:
                    nc.vector.tensor_copy(o_sb, o_ps)
                vt["dve"] += 100.0
                nc.vector.tensor_copy(den_all[:, ts(t, QCH)], db_ps[0:1, :])
                nc.sync.dma_start(outT[:, ts(t, QCH)], o_sb)

            # ================= chunk 0: bf16 path =================
            o_ps = po_pool.tile([P, QCH], F32, tag="o")
            db_ps = pd_pool.tile([P, QCH], F32, tag="db")
            qacc = spool.tile([P, QCH], BF16, tag="qacc")
            s0 = [None] * 4
            for h in range(2):
                slot = ps_pool.tile([P, 2, QCH], F32, tag="s",
                                    name=f"s0_{h}")
                s0[2 * h] = slot[:, 0, :]
                s0[2 * h + 1] = slot[:, 1, :]
            for j in range(4):
                off = j * P
                nc.tensor.matmul(s0[j][:, ds(off, QCH - off)],
                                 kT0_sb[:, ts(j, P)],
                                 qT0_sb[:, ds(off, QCH - off)],
                                 start=True, stop=True)
            for j in range(4):
                off = j * P
                nc.scalar.activation(e0[:, j, ds(off, QCH - off)],
                                     s0[j][:, ds(off, QCH - off)],
                                     EXP, scale=SCALE)
                vt["act"] += (QCH - off) * ACT_R + ACT_O
                reg = e0[:, j, ds(off, P)]
                nc.gpsimd.tensor_mul(reg, reg, tri0)
                vt["pool"] += POOL_MASK
            for j in range(4):
                off = j * P
                nc.tensor.matmul(o_ps[:, ds(off, QCH - off)],
                                 v0_sb[:, j, :],
                                 e0[:, j, ds(off, QCH - off)],
                                 start=(j == 0), stop=(j == 3))
            nc.vector.tensor_add(qacc, e0[:, 0, :], e0[:, 1, :])
            nc.vector.tensor_add(qacc, qacc, e0[:, 2, :])
            nc.vector.tensor_add(qacc, qacc, e0[:, 3, :])
            vt["dve"] += 1500.0
            nc.tensor.matmul(db_ps, ones_bf, qacc, start=True, stop=True)
            flush_chunk(0, o_ps, db_ps)

            # ================= chunks 1..7: fp8 path =================
            pv_queue = []

            den_batch = []

            def emit_den(flush):
                # dens batched in adjacent pairs: consecutive matmuls with
                # the same (ones8) weights skip the weight-reload bubble
                if len(den_batch) >= 2 or (flush and den_batch):
                    for (t, p, npair, offp, e_pair, db_ps) in den_batch:
                        w = QCH - offp
                        nc.tensor.matmul(db_ps[:, ds(offp, w)],
                                         ones8,
                                         e_pair[:, :, ds(offp, w)],
                                         start=(p == 0),
                                         stop=(p == npair - 1),
                                         perf_mode=DR)
                    den_batch.clear()

            def emit_pv(job):
                t, p, npair, offp, e_pair, o_ps, db_ps = job
                w = QCH - offp
                nc.tensor.matmul(o_ps[:, ds(offp, w)],
                                 v8_sb[:, ds(2 * p, 2), :],
                                 e_pair[:, :, ds(offp, w)],
                                 start=(p == 0), stop=(p == npair - 1),
                                 perf_mode=DR)
                den_batch.append((t, p, npair, offp, e_pair, db_ps))
                emit_den(p == npair - 1)
                if p == npair - 1:
                    flush_chunk(t, o_ps, db_ps)

            for t in range(1, NCH):
                if t <= 2:
                    # PE idle-fillers during the head DMA waits; they write
                    # the long-dead warm_db tile so the scheduler is free
                    # to slot them into the idle window
                    for _ in range(18 if t == 1 else 12):
                        nc.tensor.matmul(warm_db[:, ds(0, P)], ones_bf,
                                         ones_bf, start=True, stop=True)
                nj = 4 * (t + 1)
                npair = nj // 2
                o_ps = po_pool.tile([P, QCH], F32, tag="o")
                db_ps = pd_pool.tile([P, QCH], F32, tag="db")
                for p in range(npair):
                    j0, j1 = 2 * p, 2 * p + 1
                    dd0, dd1 = j0 - 4 * t, j1 - 4 * t
                    offp = max(dd0, 0) * P
                    s_ps = ps_pool.tile([P, 2, QCH], F32, tag="s")
                    if dd0 == 0:
                        e_pair = ediagA
                    elif dd0 == 2:
                        e_pair = ediagB
                    else:
                        e_pair = epool.tile([P, 2, QCH], F8, tag="e")
                    for i, (j, dd) in enumerate(((j0, dd0), (j1, dd1))):
                        off = max(dd, 0) * P
                        w = QCH - off
                        nc.tensor.matmul(
                            s_ps[:, i, ds(off, w)],
                            kT8_sb[:, ts(j, P)],
                            qT8_sb[:, ds(t * QCH + off, w)],
                            start=True, stop=True)
                    if dd1 < 0:
                        # non-diagonal pair: one fused instruction over
                        # [P, 2, QCH] (1024 free columns)
                        e_u8 = e_pair[:, :, :].bitcast(U8)
                        eng = pick2(2 * QCH * ACT_R + ACT_O,
                                    2 * QCH * DVE_R + DVE_O)
                        if eng == "act":
                            nc.scalar.activation(e_u8, s_ps[:, :, :], COPY,
                                                 bias=C2, scale=C1)
                        else:
                            nc.vector.tensor_scalar(e_u8, s_ps[:, :, :],
                                                    C1, C2, MULT, ADD)
                    else:
                        # diagonal pair: per-tile narrowed exp; either
                        # DVE STT (mask folded into the additive operand)
                        # or ACT Copy + Pool byte-mask on the triangle
                        for i, (j, dd) in enumerate(((j0, dd0), (j1, dd1))):
                            off = max(dd, 0) * P
                            w = QCH - off
                            e_u8 = e_pair[:, i, ds(off, w)].bitcast(U8)
                            s_in = s_ps[:, i, ds(off, w)]
                            act_c = w * ACT_R + ACT_O
                            dve_c = w * DVE_R + DVE_O
                            if (vt["act"] + act_c + 0.3 * POOL_MASK
                                    <= vt["dve"] + dve_c):
                                vt["act"] += act_c
                                vt["pool"] += POOL_MASK
                                nc.scalar.activation(e_u8, s_in, COPY,
                                                     bias=C2, scale=C1)
                                treg = e_pair[:, i, ds(off, P)]
                                nc.gpsimd.tensor_mul(treg, treg, tri_f8)
                            else:
                                vt["dve"] += dve_c
                                nc.vector.scalar_tensor_tensor(
                                    e_u8, s_in, C1, maskbias[:, :w],
                                    MULT, ADD)
                    pv_queue.append((t, p, npair, offp, e_pair, o_ps,
                                     db_ps))
                    if len(pv_queue) > 3:
                        emit_pv(pv_queue.pop(0))

            for job in pv_queue:
                emit_pv(job)

    nc.compile()
    return nc


def _get_nc():
    if not _nc_cache:
        _nc_cache.append(_build())
    return _nc_cache[0]


def _prepare_inputs(query, key, value):
    B, H, W, C = query.shape
    n = H * W
    f8 = ml_dtypes.float8_e4m3
    bf = ml_dtypes.bfloat16
    q = np.asarray(query, np.float32).reshape(B, n, C)
    k = np.asarray(key, np.float32).reshape(B, n, C)
    v = np.asarray(value, np.float32).reshape(B, n, C)
    qT = np.ascontiguousarray(q.transpose(0, 2, 1))               # [B, C, n]
    kT = np.ascontiguousarray(k.transpose(0, 2, 1))
    qT8 = qT.astype(f8)
    kT8 = kT.astype(f8)
    qT0 = qT[:, :, :QCH].astype(bf)
    kT0 = kT[:, :, :QCH].astype(bf)
    # v8 [128, n]: v8[p, j*128 + c] = v[j*128 + p, c]
    v8 = np.ascontiguousarray(
        v.reshape(B, n // P, P, C).transpose(0, 2, 1, 3).reshape(B, P, n)
    ).astype(f8)
    v0 = np.ascontiguousarray(
        v[:, :QCH].reshape(B, 4, P, C).transpose(0, 2, 1, 3).reshape(B, P, 4 * C)
    ).astype(bf)
    return [
        {"qT8": qT8[b], "kT8": kT8[b], "qT0": qT0[b], "kT0": kT0[b],
         "v8": v8[b], "v0": v0[b]}
        for b in range(B)
    ]


def kernel(query, key, value):
    B, H, W, C = query.shape
    CV = value.shape[-1]
    n = H * W
    in_maps = _prepare_inputs(query, key, value)
    nc = _get_nc()
    res = run_bass_kernel_spmd(nc, in_maps, core_ids=list(range(N_CORES)))
    out = np.empty((B, n, CV), np.float32)
    for b in range(B):
        oT = res.results[b]["outT"].astype(np.float32)   # [128, n]
        dn = res.results[b]["den"]                       # [1, n]
        out[b] = (oT / dn).T
    return out.reshape(B, H, W, CV)


# revision 25
# speedup vs baseline: 1.0040x; 1.0040x over previous
"""Causal attention (B=8, N=4096 flattened 64x64, d=128) on 8 trn2 cores.

Sharding: data-parallel over batch -- core b gets batch element b.

Per-core algorithm, transposed orientation (S^T[k, q] tiles, O^T[c, q]):

  fp8 fast path (q-chunks 1..7, i.e. rows with >= 512 causal keys):
    - Q/K/V quantized to fp8e4m3 on host. QK^T and PV/den matmuls run in
      DoubleRow perf mode (0.5 PE cycles/output column): the d=128
      contraction is split as [64 partitions x 2 subtiles] for S, and
      k-tiles are processed in adjacent PAIRS ([128, 2, *]) for PV/den.
    - exp via the "Schraudolph byte" trick: the fp8e4m3 bit pattern of
      exp(s/sqrt(128)) is byte(s*C1 + C2) with C1 = 8*log2(e)/sqrt(128),
      C2 = 56 (= 7*8, the e4m3 exponent bias in 1/8-octave units).
      tensor_scalar/activation(Copy) compute y = s*C1 + C2 and the
      fp32->uint8 output conversion (round-nearest, SATURATING at 0/255 --
      verified on HW) yields E = exp(.) directly as fp8 bytes. Saturation
      at 0 zeroes masked/tiny entries; 255 needs a +17 sigma score (never).
      The same formula runs on ACT (activation Copy), DVE and Pool
      (tensor_scalar), statically load-balanced across the three.
    - causal diagonal tiles: scalar_tensor_tensor folds the triangle mask
      into the additive operand (C2 on allowed cells, -1000 on masked), so
      masking costs nothing extra. Diagonal E tiles live in dedicated
      buffers whose never-written prefix is zeroed once at init.
    - denominator: ones8^T @ E pair-matmuls accumulate in PSUM (no DVE
      reduction chain at all).

  bf16 head path (q-chunk 0): rows with < 512 keys have too few terms for
  fp8 weight noise to average out, so chunk 0 runs the baseline-style
  bf16 pipeline (true exp on ACT -> bf16 E, bf16 V). Costs ~3us.

  outputs per core: outT [128, 4096] fp16 (unnormalized O^T), den [1,4096]
  fp32; host computes (outT / den).T in fp32.
"""

import math

import ml_dtypes
import numpy as np

import concourse.bacc as bacc
import concourse.mybir as mybir
import concourse.tile as tile
from concourse.bass import ts, ds
from concourse.bass_utils import run_bass_kernel_spmd

P = 128
NSEQ = 4096
QCH = 512              # query positions per chunk
NCH = NSEQ // QCH      # 8 chunks
N_CORES = 8
SCALE = 1.0 / math.sqrt(128.0)
C1 = 8.0 / math.log(2.0) / math.sqrt(128.0)   # score -> byte slope
C2 = 40.0   # e4m3 exponent bias*8 (56) minus 16: scales all E by 2^-2
            # (softmax-invariant) so byte 120 (=inf in IEEE e4m3) needs a
            # +6.9 sigma score instead of +5.6 (which does occur)
NEGB = -100000.0                              # masked-cell additive bias

F32 = mybir.dt.float32
F16 = mybir.dt.float16
BF16 = mybir.dt.bfloat16
F8 = mybir.dt.float8e4
U8 = mybir.dt.uint8
DR = mybir.MatmulPerfMode.DoubleRow
MULT = mybir.AluOpType.mult
ADD = mybir.AluOpType.add
EXP = mybir.ActivationFunctionType.Exp
COPY = mybir.ActivationFunctionType.Copy

_nc_cache = []


def _build():
    nc = bacc.Bacc("TRN2", target_bir_lowering=False, debug=False,
                   num_devices=N_CORES)
    qT8 = nc.dram_tensor("qT8", [P, NSEQ], F8, kind="ExternalInput").ap()
    kT8 = nc.dram_tensor("kT8", [P, NSEQ], F8, kind="ExternalInput").ap()
    qT0 = nc.dram_tensor("qT0", [P, QCH], BF16, kind="ExternalInput").ap()
    kT0 = nc.dram_tensor("kT0", [P, QCH], BF16, kind="ExternalInput").ap()
    v8 = nc.dram_tensor("v8", [P, NSEQ], F8, kind="ExternalInput").ap()
    v0 = nc.dram_tensor("v0", [P, 4 * P], BF16, kind="ExternalInput").ap()
    outT = nc.dram_tensor("outT", [P, NSEQ], F16, kind="ExternalOutput").ap()
    den = nc.dram_tensor("den", [1, NSEQ], F32, kind="ExternalOutput").ap()

    # ---- static engine schedule for the exp work --------------------
    # virtual-time greedy balancing; costs in ns per whole instruction.
    # (Pool/gpsimd cannot touch PSUM, so exp runs on ACT+DVE only; Pool
    # handles SBUF-side byte masks for ACT-routed diagonal tiles.)
    vt = {"act": 0.0, "dve": 0.0, "pool": 0.0}
    ACT_R, ACT_O = 0.68, 217.0
    DVE_R, DVE_O = 1.19, 230.0
    POOL_MASK = 480.0

    def pick2(act_cost, dve_cost):
        if vt["act"] + act_cost <= vt["dve"] + dve_cost:
            vt["act"] += act_cost
            return "act"
        vt["dve"] += dve_cost
        return "dve"

    with tile.TileContext(nc) as tc:
        with (
            tc.tile_pool(name="const", bufs=1) as cpool,
            tc.tile_pool(name="epool", bufs=6) as epool,
            tc.tile_pool(name="spool", bufs=2) as spool,
            tc.tile_pool(name="ps_s", bufs=3, space="PSUM") as ps_pool,
            tc.tile_pool(name="ps_o", bufs=1, space="PSUM") as po_pool,
            tc.tile_pool(name="ps_d", bufs=1, space="PSUM") as pd_pool,
        ):
            # ---- constants ----
            ones_bf = cpool.tile([P, P], BF16)
            nc.gpsimd.memset(ones_bf, 1.0)
            ones8 = cpool.tile([P, 2, P], F8)
            nc.gpsimd.memset(ones8, 1.0)
            # maskbias [P, QCH] f32: C2 everywhere except the leading
            # [P, P] triangle block where masked (q' < k) cells get NEGB
            maskbias = cpool.tile([P, QCH], F32)
            nc.vector.memset(maskbias, C2)
            nc.gpsimd.affine_select(
                out=maskbias[:, :P], in_=maskbias[:, :P],
                compare_op=mybir.AluOpType.is_ge, fill=NEGB,
                base=0, pattern=[[1, P]], channel_multiplier=-1)
            # tri0 [P, P] bf16: 1 where q' >= k else 0 (chunk-0 masks)
            scratch = cpool.tile([P, P], F32)
            nc.gpsimd.memset(scratch, 1.0)
            nc.gpsimd.affine_select(
                out=scratch, in_=scratch,
                compare_op=mybir.AluOpType.is_ge, fill=0.0,
                base=0, pattern=[[1, P]], channel_multiplier=-1)
            tri0 = cpool.tile([P, P], BF16)
            nc.vector.tensor_copy(tri0, scratch)
            # tri_f8 [P, P] fp8: same triangle as 0.0/1.0 (fp multiply by
            # exactly 0/1 masks the fp8 E bytes losslessly)
            tri_f8 = cpool.tile([P, P], F8)
            nc.vector.tensor_copy(tri_f8, scratch)

            # prefetch the Exp activation table while DMAs run
            tl = cpool.tile([1, 1], F32)
            nc.vector.memset(tl, 0.0)
            nc.scalar.activation(tl, tl, EXP, scale=1.0)

            # PE warmup: spin the clock up during the input DMA wait;
            # chunk 0's first den matmul (start=True) clears this bank
            warm_db = pd_pool.tile([P, QCH], F32, tag="db", name="warm")
            for _ in range(26):
                nc.tensor.matmul(warm_db[:, ds(0, P)], ones_bf, ones_bf,
                                 start=True, stop=True)

            # ---- input DMAs ----
            # sync queue: chunk-0 bf16 head then the K fp8 stream;
            # scalar queue: V + Q fp8 stream; pieces in first-use order
            v0_sb = cpool.tile([P, 4, P], BF16)
            qT0_sb = cpool.tile([P, QCH], BF16)
            kT0_sb = cpool.tile([P, QCH], BF16)
            qT8_sb = cpool.tile([P, NSEQ], F8)
            kT8_sb = cpool.tile([P, NSEQ], F8)
            v8_sb = cpool.tile([P, NSEQ // P, P], F8)
            nc.sync.dma_start(kT0_sb, kT0)
            nc.sync.dma_start(kT8_sb[:, :2 * QCH], kT8[:, :2 * QCH])
            nc.sync.dma_start(kT8_sb[:, 2 * QCH:4 * QCH],
                              kT8[:, 2 * QCH:4 * QCH])
            nc.sync.dma_start(kT8_sb[:, 4 * QCH:], kT8[:, 4 * QCH:])
            nc.scalar.dma_start(qT0_sb, qT0)
            nc.scalar.dma_start(v0_sb, v0.rearrange("p (j c) -> p j c", c=P))
            nc.scalar.dma_start(qT8_sb[:, :2 * QCH], qT8[:, :2 * QCH])
            nc.scalar.dma_start(
                v8_sb[:, :8, :],
                v8[:, :8 * P].rearrange("p (j c) -> p j c", c=P))
            nc.scalar.dma_start(qT8_sb[:, 2 * QCH:4 * QCH],
                                qT8[:, 2 * QCH:4 * QCH])
            nc.scalar.dma_start(
                v8_sb[:, 8:, :],
                v8[:, 8 * P:].rearrange("p (j c) -> p j c", c=P))
            nc.scalar.dma_start(qT8_sb[:, 4 * QCH:], qT8[:, 4 * QCH:])

            # dedicated diagonal E buffers (never-written prefixes stay 0)
            ediagA = cpool.tile([P, 2, QCH], F8)   # dd = 0, 1
            ediagB = cpool.tile([P, 2, QCH], F8)   # dd = 2, 3
            nc.vector.memset(ediagA.bitcast(U8), 0)
            nc.vector.memset(ediagB.bitcast(U8), 0)
            e0 = cpool.tile([P, 4, QCH], BF16)     # chunk-0 E (bf16)
            nc.gpsimd.memset(e0, 0.0)

            den_all = cpool.tile([1, NSEQ], F32)

            def flush_chunk(t, o_ps, db_ps):
                o_sb = spool.tile([P, QCH], F16, tag="osb", name=f"osb{t}")
                if t == NCH - 1:
                    # tail: halve latency -- split the copy across ACT+DVE
                    # and DMA each half as soon as it lands
                    h = QCH // 2
                    nc.vector.tensor_copy(den_all[:, ts(t, QCH)],
                                          db_ps[0:1, :])
                    nc.sync.dma_start(den, den_all)
                    nc.scalar.copy(o_sb[:, :h], o_ps[:, :h])
                    nc.sync.dma_start(outT[:, ds(t * QCH, h)], o_sb[:, :h])
                    nc.vector.tensor_copy(o_sb[:, h:], o_ps[:, h:])
                    nc.sync.dma_start(outT[:, ds(t * QCH + h, h)],
                                      o_sb[:, h:])
                    return
                eng = pick2(QCH * ACT_R + ACT_O, QCH * DVE_R + DVE_O)
                if eng == "act":
                    nc.scalar.copy(o_sb, o_ps)
                else:
                    nc.vector.tensor_copy(o_sb, o_ps)
                vt["dve"] += 100.0
                nc.vector.tensor_copy(den_all[:, ts(t, QCH)], db_ps[0:1, :])
                nc.sync.dma_start(outT[:, ts(t, QCH)], o_sb)

            # ================= chunk 0: bf16 path =================
            o_ps = po_pool.tile([P, QCH], F32, tag="o")
            db_ps = pd_pool.tile([P, QCH], F32, tag="db")
            qacc = spool.tile([P, QCH], BF16, tag="qacc")
            s0 = [None] * 4
            for h in range(2):
                slot = ps_pool.tile([P, 2, QCH], F32, tag="s",
                                    name=f"s0_{h}")
                s0[2 * h] = slot[:, 0, :]
                s0[2 * h + 1] = slot[:, 1, :]
            for j in range(4):
                off = j * P
                nc.tensor.matmul(s0[j][:, ds(off, QCH - off)],
                                 kT0_sb[:, ts(j, P)],
                                 qT0_sb[:, ds(off, QCH - off)],
                                 start=True, stop=True)
            for j in range(4):
                off = j * P
                nc.scalar.activation(e0[:, j, ds(off, QCH - off)],
                                     s0[j][:, ds(off, QCH - off)],
                                     EXP, scale=SCALE)
                vt["act"] += (QCH - off) * ACT_R + ACT_O
                reg = e0[:, j, ds(off, P)]
                nc.gpsimd.tensor_mul(reg, reg, tri0)
                vt["pool"] += POOL_MASK
            for j in range(4):
                off = j * P
                nc.tensor.matmul(o_ps[:, ds(off, QCH - off)],
                                 v0_sb[:, j, :],
                                 e0[:, j, ds(off, QCH - off)],
                                 start=(j == 0), stop=(j == 3))
            nc.vector.tensor_add(qacc, e0[:, 0, :], e0[:, 1, :])
            nc.vector.tensor_add(qacc, qacc, e0[:, 2, :])
            nc.vector.tensor_add(qacc, qacc, e0[:, 3, :])
            vt["dve"] += 1500.0
            nc.tensor.matmul(db_ps, ones_bf, qacc, start=True, stop=True)
            flush_chunk(0, o_ps, db_ps)

            # ================= chunks 1..7: fp8 path =================
            pv_queue = []

            den_batch = []

            def emit_den(flush):
                # dens batched in adjacent pairs: consecutive matmuls with
                # the same (ones8) weights skip the weight-reload bubble
                if len(den_batch) >= 2 or (flush and den_batch):
                    for (t, p, npair, offp, e_pair, db_ps) in den_batch:
                        w = QCH - offp
                        nc.tensor.matmul(db_ps[:, ds(offp, w)],
                                         ones8,
                                         e_pair[:, :, ds(offp, w)],
                                         start=(p == 0),
                                         stop=(p == npair - 1),
                                         perf_mode=DR)
                    den_batch.clear()

            def emit_pv(job):
                t, p, npair, offp, e_pair, o_ps, db_ps = job
                w = QCH - offp
                nc.tensor.matmul(o_ps[:, ds(offp, w)],
                                 v8_sb[:, ds(2 * p, 2), :],
                                 e_pair[:, :, ds(offp, w)],
                                 start=(p == 0), stop=(p == npair - 1),
                                 perf_mode=DR)
                den_batch.append((t, p, npair, offp, e_pair, db_ps))
                emit_den(p == npair - 1)
                if p == npair - 1:
                    flush_chunk(t, o_ps, db_ps)

            for t in range(1, NCH):
                if t <= 2:
                    # PE idle-fillers during the head DMA waits; they write
                    # the long-dead warm_db tile so the scheduler is free
                    # to slot them into the idle window
                    for _ in range(12 if t == 1 else 6):
                        nc.tensor.matmul(warm_db[:, ds(0, P)], ones_bf,
                                         ones_bf, start=True, stop=True)
                nj = 4 * (t + 1)
                npair = nj // 2
                o_ps = po_pool.tile([P, QCH], F32, tag="o")
                db_ps = pd_pool.tile([P, QCH], F32, tag="db")
                for p in range(npair):
                    j0, j1 = 2 * p, 2 * p + 1
                    dd0, dd1 = j0 - 4 * t, j1 - 4 * t
                    offp = max(dd0, 0) * P
                    s_ps = ps_pool.tile([P, 2, QCH], F32, tag="s")
                    if dd0 == 0:
                        e_pair = ediagA
                    elif dd0 == 2:
                        e_pair = ediagB
                    else:
                        e_pair = epool.tile([P, 2, QCH], F8, tag="e")
                    for i, (j, dd) in enumerate(((j0, dd0), (j1, dd1))):
                        off = max(dd, 0) * P
                        w = QCH - off
                        nc.tensor.matmul(
                            s_ps[:, i, ds(off, w)],
                            kT8_sb[:, ts(j, P)],
                            qT8_sb[:, ds(t * QCH + off, w)],
                            start=True, stop=True)
                    if dd1 < 0:
                        # non-diagonal pair: one fused instruction over
                        # [P, 2, QCH] (1024 free columns)
                        e_u8 = e_pair[:, :, :].bitcast(U8)
                        eng = pick2(2 * QCH * ACT_R + ACT_O,
                                    2 * QCH * DVE_R + DVE_O)
                        if eng == "act":
                            nc.scalar.activation(e_u8, s_ps[:, :, :], COPY,
                                                 bias=C2, scale=C1)
                        else:
                            nc.vector.tensor_scalar(e_u8, s_ps[:, :, :],
                                                    C1, C2, MULT, ADD)
                    else:
                        # diagonal pair: per-tile narrowed exp; either
                        # DVE STT (mask folded into the additive operand)
                        # or ACT Copy + Pool byte-mask on the triangle
                        for i, (j, dd) in enumerate(((j0, dd0), (j1, dd1))):
                            off = max(dd, 0) * P
                            w = QCH - off
                            e_u8 = e_pair[:, i, ds(off, w)].bitcast(U8)
                            s_in = s_ps[:, i, ds(off, w)]
                            act_c = w * ACT_R + ACT_O
                            dve_c = w * DVE_R + DVE_O
                            if (vt["act"] + act_c + 0.3 * POOL_MASK
                                    <= vt["dve"] + dve_c):
                                vt["act"] += act_c
                                vt["pool"] += POOL_MASK
                                nc.scalar.activation(e_u8, s_in, COPY,
                                                     bias=C2, scale=C1)
                                treg = e_pair[:, i, ds(off, P)]
                                nc.gpsimd.tensor_mul(treg, treg, tri_f8)
                            else:
                                vt["dve"] += dve_c
                                nc.vector.scalar_tensor_tensor(
                                    e_u8, s_in, C1, maskbias[:, :w],
                                    MULT, ADD)
                    pv_queue.append((t, p, npair, offp, e_pair, o_ps,
                                     db_ps))
                    if len(pv_queue) > 3:
                        emit_pv(pv_queue.pop(0))

            for job in pv_queue:
                emit_pv(job)

    nc.compile()
    return nc


def _get_nc():
    if not _nc_cache:
        _nc_cache.append(_build())
    return _nc_cache[0]


def _prepare_inputs(query, key, value):
    B, H, W, C = query.shape
    n = H * W
    f8 = ml_dtypes.float8_e4m3
    bf = ml_dtypes.bfloat16
    q = np.asarray(query, np.float32).reshape(B, n, C)
    k = np.asarray(key, np.float32).reshape(B, n, C)
    v = np.asarray(value, np.float32).reshape(B, n, C)
    qT = np.ascontiguousarray(q.transpose(0, 2, 1))               # [B, C, n]
    kT = np.ascontiguousarray(k.transpose(0, 2, 1))
    qT8 = qT.astype(f8)
    kT8 = kT.astype(f8)
    qT0 = qT[:, :, :QCH].astype(bf)
    kT0 = kT[:, :, :QCH].astype(bf)
    # v8 [128, n]: v8[p, j*128 + c] = v[j*128 + p, c]
    v8 = np.ascontiguousarray(
        v.reshape(B, n // P, P, C).transpose(0, 2, 1, 3).reshape(B, P, n)
    ).astype(f8)
    v0 = np.ascontiguousarray(
        v[:, :QCH].reshape(B, 4, P, C).transpose(0, 2, 1, 3).reshape(B, P, 4 * C)
    ).astype(bf)
    return [
        {"qT8": qT8[b], "kT8": kT8[b], "qT0": qT0[b], "kT0": kT0[b],
         "v8": v8[b], "v0": v0[b]}
        for b in range(B)
    ]


def kernel(query, key, value):
    B, H, W, C = query.shape
    CV = value.shape[-1]
    n = H * W
    in_maps = _prepare_inputs(query, key, value)
    nc = _get_nc()
    res = run_bass_kernel_spmd(nc, in_maps, core_ids=list(range(N_CORES)))
    out = np.empty((B, n, CV), np.float32)
    for b in range(B):
        oT = res.results[b]["outT"].astype(np.float32)   # [128, n]
        dn = res.results[b]["den"]                       # [1, n]
        out[b] = (oT / dn).T
    return out.reshape(B, H, W, CV)


# revision 26
# speedup vs baseline: 1.0257x; 1.0216x over previous
"""Causal attention (B=8, N=4096 flattened 64x64, d=128) on 8 trn2 cores.

Sharding: data-parallel over batch -- core b gets batch element b.

Per-core algorithm, transposed orientation (S^T[k, q] tiles, O^T[c, q]):

  fp8 fast path (q-chunks 1..7, i.e. rows with >= 512 causal keys):
    - Q/K/V quantized to fp8e4m3 on host. QK^T and PV/den matmuls run in
      DoubleRow perf mode (0.5 PE cycles/output column): the d=128
      contraction is split as [64 partitions x 2 subtiles] for S, and
      k-tiles are processed in adjacent PAIRS ([128, 2, *]) for PV/den.
    - exp via the "Schraudolph byte" trick: the fp8e4m3 bit pattern of
      exp(s/sqrt(128)) is byte(s*C1 + C2) with C1 = 8*log2(e)/sqrt(128),
      C2 = 56 (= 7*8, the e4m3 exponent bias in 1/8-octave units).
      tensor_scalar/activation(Copy) compute y = s*C1 + C2 and the
      fp32->uint8 output conversion (round-nearest, SATURATING at 0/255 --
      verified on HW) yields E = exp(.) directly as fp8 bytes. Saturation
      at 0 zeroes masked/tiny entries; 255 needs a +17 sigma score (never).
      The same formula runs on ACT (activation Copy), DVE and Pool
      (tensor_scalar), statically load-balanced across the three.
    - causal diagonal tiles: scalar_tensor_tensor folds the triangle mask
      into the additive operand (C2 on allowed cells, -1000 on masked), so
      masking costs nothing extra. Diagonal E tiles live in dedicated
      buffers whose never-written prefix is zeroed once at init.
    - denominator: ones8^T @ E pair-matmuls accumulate in PSUM (no DVE
      reduction chain at all).

  bf16 head path (q-chunk 0): rows with < 512 keys have too few terms for
  fp8 weight noise to average out, so chunk 0 runs the baseline-style
  bf16 pipeline (true exp on ACT -> bf16 E, bf16 V). Costs ~3us.

  outputs per core: outT [128, 4096] fp16 (unnormalized O^T), den [1,4096]
  fp32; host computes (outT / den).T in fp32.
"""

import math

import ml_dtypes
import numpy as np

import concourse.bacc as bacc
import concourse.mybir as mybir
import concourse.tile as tile
from concourse.bass import ts, ds
from concourse.bass_utils import run_bass_kernel_spmd

P = 128
NSEQ = 4096
QCH = 512              # query positions per chunk
NCH = NSEQ // QCH      # 8 chunks
N_CORES = 8
SCALE = 1.0 / math.sqrt(128.0)
C1 = 8.0 / math.log(2.0) / math.sqrt(128.0)   # score -> byte slope
C2 = 40.0   # e4m3 exponent bias*8 (56) minus 16: scales all E by 2^-2
            # (softmax-invariant) so byte 120 (=inf in IEEE e4m3) needs a
            # +6.9 sigma score instead of +5.6 (which does occur)
NEGB = -100000.0                              # masked-cell additive bias

F32 = mybir.dt.float32
F16 = mybir.dt.float16
BF16 = mybir.dt.bfloat16
F8 = mybir.dt.float8e4
U8 = mybir.dt.uint8
DR = mybir.MatmulPerfMode.DoubleRow
MULT = mybir.AluOpType.mult
ADD = mybir.AluOpType.add
EXP = mybir.ActivationFunctionType.Exp
COPY = mybir.ActivationFunctionType.Copy

_nc_cache = []


def _build():
    nc = bacc.Bacc("TRN2", target_bir_lowering=False, debug=False,
                   num_devices=N_CORES)
    qT8 = nc.dram_tensor("qT8", [P, NSEQ], F8, kind="ExternalInput").ap()
    kT8 = nc.dram_tensor("kT8", [P, NSEQ], F8, kind="ExternalInput").ap()
    qT0 = nc.dram_tensor("qT0", [P, QCH], BF16, kind="ExternalInput").ap()
    kT0 = nc.dram_tensor("kT0", [P, QCH], BF16, kind="ExternalInput").ap()
    v8 = nc.dram_tensor("v8", [P, NSEQ], F8, kind="ExternalInput").ap()
    v0 = nc.dram_tensor("v0", [P, 4 * P], BF16, kind="ExternalInput").ap()
    outT = nc.dram_tensor("outT", [P, NSEQ], F16, kind="ExternalOutput").ap()
    den = nc.dram_tensor("den", [1, NSEQ], F32, kind="ExternalOutput").ap()

    # ---- static engine schedule for the exp work --------------------
    # virtual-time greedy balancing; costs in ns per whole instruction.
    # (Pool/gpsimd cannot touch PSUM, so exp runs on ACT+DVE only; Pool
    # handles SBUF-side byte masks for ACT-routed diagonal tiles.)
    vt = {"act": 0.0, "dve": 0.0, "pool": 0.0}
    ACT_R, ACT_O = 0.68, 217.0
    DVE_R, DVE_O = 1.19, 230.0
    POOL_MASK = 480.0

    def pick2(act_cost, dve_cost):
        if vt["act"] + act_cost <= vt["dve"] + dve_cost:
            vt["act"] += act_cost
            return "act"
        vt["dve"] += dve_cost
        return "dve"

    with tile.TileContext(nc) as tc:
        with (
            tc.tile_pool(name="const", bufs=1) as cpool,
            tc.tile_pool(name="epool", bufs=6) as epool,
            tc.tile_pool(name="spool", bufs=2) as spool,
            tc.tile_pool(name="ps_s", bufs=3, space="PSUM") as ps_pool,
            tc.tile_pool(name="ps_o", bufs=1, space="PSUM") as po_pool,
            tc.tile_pool(name="ps_d", bufs=1, space="PSUM") as pd_pool,
        ):
            # ---- constants ----
            ones_bf = cpool.tile([P, P], BF16)
            nc.gpsimd.memset(ones_bf, 1.0)
            ones8 = cpool.tile([P, 2, P], F8)
            nc.gpsimd.memset(ones8, 1.0)
            # maskbias [P, QCH] f32: C2 everywhere except the leading
            # [P, P] triangle block where masked (q' < k) cells get NEGB
            maskbias = cpool.tile([P, QCH], F32)
            nc.vector.memset(maskbias, C2)
            nc.gpsimd.affine_select(
                out=maskbias[:, :P], in_=maskbias[:, :P],
                compare_op=mybir.AluOpType.is_ge, fill=NEGB,
                base=0, pattern=[[1, P]], channel_multiplier=-1)
            # tri0 [P, P] bf16: 1 where q' >= k else 0 (chunk-0 masks)
            scratch = cpool.tile([P, P], F32)
            nc.gpsimd.memset(scratch, 1.0)
            nc.gpsimd.affine_select(
                out=scratch, in_=scratch,
                compare_op=mybir.AluOpType.is_ge, fill=0.0,
                base=0, pattern=[[1, P]], channel_multiplier=-1)
            tri0 = cpool.tile([P, P], BF16)
            nc.vector.tensor_copy(tri0, scratch)
            # tri_f8 [P, P] fp8: same triangle as 0.0/1.0 (fp multiply by
            # exactly 0/1 masks the fp8 E bytes losslessly)
            tri_f8 = cpool.tile([P, P], F8)
            nc.vector.tensor_copy(tri_f8, scratch)

            # prefetch the Exp activation table while DMAs run
            tl = cpool.tile([1, 1], F32)
            nc.vector.memset(tl, 0.0)
            nc.scalar.activation(tl, tl, EXP, scale=1.0)

            # PE warmup: spin the clock up during the input DMA wait;
            # chunk 0's first den matmul (start=True) clears this bank
            warm_db = pd_pool.tile([P, QCH], F32, tag="db", name="warm")
            for _ in range(26):
                nc.tensor.matmul(warm_db[:, ds(0, P)], ones_bf, ones_bf,
                                 start=True, stop=True)

            # ---- input DMAs ----
            # sync queue: chunk-0 bf16 head then the K fp8 stream;
            # scalar queue: V + Q fp8 stream; pieces in first-use order
            v0_sb = cpool.tile([P, 4, P], BF16)
            qT0_sb = cpool.tile([P, QCH], BF16)
            kT0_sb = cpool.tile([P, QCH], BF16)
            qT8_sb = cpool.tile([P, NSEQ], F8)
            kT8_sb = cpool.tile([P, NSEQ], F8)
            v8_sb = cpool.tile([P, NSEQ // P, P], F8)
            nc.sync.dma_start(kT0_sb, kT0)
            nc.sync.dma_start(kT8_sb[:, :2 * QCH], kT8[:, :2 * QCH])
            nc.sync.dma_start(kT8_sb[:, 2 * QCH:4 * QCH],
                              kT8[:, 2 * QCH:4 * QCH])
            nc.sync.dma_start(kT8_sb[:, 4 * QCH:], kT8[:, 4 * QCH:])
            nc.scalar.dma_start(qT0_sb, qT0)
            nc.scalar.dma_start(v0_sb, v0.rearrange("p (j c) -> p j c", c=P))
            nc.scalar.dma_start(qT8_sb[:, :2 * QCH], qT8[:, :2 * QCH])
            nc.scalar.dma_start(
                v8_sb[:, :8, :],
                v8[:, :8 * P].rearrange("p (j c) -> p j c", c=P))
            nc.scalar.dma_start(qT8_sb[:, 2 * QCH:4 * QCH],
                                qT8[:, 2 * QCH:4 * QCH])
            nc.scalar.dma_start(
                v8_sb[:, 8:, :],
                v8[:, 8 * P:].rearrange("p (j c) -> p j c", c=P))
            nc.scalar.dma_start(qT8_sb[:, 4 * QCH:], qT8[:, 4 * QCH:])

            # dedicated diagonal E buffers (never-written prefixes stay 0)
            ediagA = cpool.tile([P, 2, QCH], F8)   # dd = 0, 1
            ediagB = cpool.tile([P, 2, QCH], F8)   # dd = 2, 3
            nc.vector.memset(ediagA.bitcast(U8), 0)
            nc.vector.memset(ediagB.bitcast(U8), 0)
            e0 = cpool.tile([P, 4, QCH], BF16)     # chunk-0 E (bf16)
            nc.gpsimd.memset(e0, 0.0)

            den_all = cpool.tile([1, NSEQ], F32)

            def flush_chunk(t, o_ps, db_ps):
                o_sb = spool.tile([P, QCH], F16, tag="osb", name=f"osb{t}")
                if t == NCH - 1:
                    eng = "act"   # tail: ACT finishes first, lowest latency
                else:
                    eng = pick2(QCH * ACT_R + ACT_O, QCH * DVE_R + DVE_O)
                if eng == "act":
                    nc.scalar.copy(o_sb, o_ps)
                else:
                    nc.vector.tensor_copy(o_sb, o_ps)
                vt["dve"] += 100.0
                nc.vector.tensor_copy(den_all[:, ts(t, QCH)], db_ps[0:1, :])
                nc.sync.dma_start(outT[:, ts(t, QCH)], o_sb)
                if t == NCH - 1:
                    nc.sync.dma_start(den, den_all)

            # ================= chunk 0: bf16 path =================
            o_ps = po_pool.tile([P, QCH], F32, tag="o")
            db_ps = pd_pool.tile([P, QCH], F32, tag="db")
            qacc = spool.tile([P, QCH], BF16, tag="qacc")
            s0 = [None] * 4
            for h in range(2):
                slot = ps_pool.tile([P, 2, QCH], F32, tag="s",
                                    name=f"s0_{h}")
                s0[2 * h] = slot[:, 0, :]
                s0[2 * h + 1] = slot[:, 1, :]
            for j in range(4):
                off = j * P
                nc.tensor.matmul(s0[j][:, ds(off, QCH - off)],
                                 kT0_sb[:, ts(j, P)],
                                 qT0_sb[:, ds(off, QCH - off)],
                                 start=True, stop=True)
            for j in range(4):
                off = j * P
                nc.scalar.activation(e0[:, j, ds(off, QCH - off)],
                                     s0[j][:, ds(off, QCH - off)],
                                     EXP, scale=SCALE)
                vt["act"] += (QCH - off) * ACT_R + ACT_O
                reg = e0[:, j, ds(off, P)]
                nc.gpsimd.tensor_mul(reg, reg, tri0)
                vt["pool"] += POOL_MASK
            for j in range(4):
                off = j * P
                nc.tensor.matmul(o_ps[:, ds(off, QCH - off)],
                                 v0_sb[:, j, :],
                                 e0[:, j, ds(off, QCH - off)],
                                 start=(j == 0), stop=(j == 3))
            nc.vector.tensor_add(qacc, e0[:, 0, :], e0[:, 1, :])
            nc.vector.tensor_add(qacc, qacc, e0[:, 2, :])
            nc.vector.tensor_add(qacc, qacc, e0[:, 3, :])
            vt["dve"] += 1500.0
            nc.tensor.matmul(db_ps, ones_bf, qacc, start=True, stop=True)
            flush_chunk(0, o_ps, db_ps)

            # ================= chunks 1..7: fp8 path =================
            pv_queue = []

            den_batch = []

            def emit_den(flush):
                # dens batched in adjacent pairs: consecutive matmuls with
                # the same (ones8) weights skip the weight-reload bubble
                if len(den_batch) >= 2 or (flush and den_batch):
                    for (t, p, npair, offp, e_pair, db_ps) in den_batch:
                        w = QCH - offp
                        nc.tensor.matmul(db_ps[:, ds(offp, w)],
                                         ones8,
                                         e_pair[:, :, ds(offp, w)],
                                         start=(p == 0),
                                         stop=(p == npair - 1),
                                         perf_mode=DR)
                    den_batch.clear()

            def emit_pv(job):
                t, p, npair, offp, e_pair, o_ps, db_ps = job
                w = QCH - offp
                nc.tensor.matmul(o_ps[:, ds(offp, w)],
                                 v8_sb[:, ds(2 * p, 2), :],
                                 e_pair[:, :, ds(offp, w)],
                                 start=(p == 0), stop=(p == npair - 1),
                                 perf_mode=DR)
                den_batch.append((t, p, npair, offp, e_pair, db_ps))
                emit_den(p == npair - 1)
                if p == npair - 1:
                    flush_chunk(t, o_ps, db_ps)

            for t in range(1, NCH):
                if t <= 2:
                    # PE idle-fillers during the head DMA waits; they write
                    # the long-dead warm_db tile so the scheduler is free
                    # to slot them into the idle window
                    for _ in range(12 if t == 1 else 6):
                        nc.tensor.matmul(warm_db[:, ds(0, P)], ones_bf,
                                         ones_bf, start=True, stop=True)
                nj = 4 * (t + 1)
                npair = nj // 2
                o_ps = po_pool.tile([P, QCH], F32, tag="o")
                db_ps = pd_pool.tile([P, QCH], F32, tag="db")
                for p in range(npair):
                    j0, j1 = 2 * p, 2 * p + 1
                    dd0, dd1 = j0 - 4 * t, j1 - 4 * t
                    offp = max(dd0, 0) * P
                    s_ps = ps_pool.tile([P, 2, QCH], F32, tag="s")
                    if dd0 == 0:
                        e_pair = ediagA
                    elif dd0 == 2:
                        e_pair = ediagB
                    else:
                        e_pair = epool.tile([P, 2, QCH], F8, tag="e")
                    for i, (j, dd) in enumerate(((j0, dd0), (j1, dd1))):
                        off = max(dd, 0) * P
                        w = QCH - off
                        nc.tensor.matmul(
                            s_ps[:, i, ds(off, w)],
                            kT8_sb[:, ts(j, P)],
                            qT8_sb[:, ds(t * QCH + off, w)],
                            start=True, stop=True)
                    if dd1 < 0:
                        # non-diagonal pair: one fused instruction over
                        # [P, 2, QCH] (1024 free columns)
                        e_u8 = e_pair[:, :, :].bitcast(U8)
                        eng = pick2(2 * QCH * ACT_R + ACT_O,
                                    2 * QCH * DVE_R + DVE_O)
                        if eng == "act":
                            nc.scalar.activation(e_u8, s_ps[:, :, :], COPY,
                                                 bias=C2, scale=C1)
                        else:
                            nc.vector.tensor_scalar(e_u8, s_ps[:, :, :],
                                                    C1, C2, MULT, ADD)
                    else:
                        # diagonal pair: per-tile narrowed exp; either
                        # DVE STT (mask folded into the additive operand)
                        # or ACT Copy + Pool byte-mask on the triangle
                        for i, (j, dd) in enumerate(((j0, dd0), (j1, dd1))):
                            off = max(dd, 0) * P
                            w = QCH - off
                            e_u8 = e_pair[:, i, ds(off, w)].bitcast(U8)
                            s_in = s_ps[:, i, ds(off, w)]
                            act_c = w * ACT_R + ACT_O
                            dve_c = w * DVE_R + DVE_O
                            if (vt["act"] + act_c + 0.3 * POOL_MASK
                                    <= vt["dve"] + dve_c):
                                vt["act"] += act_c
                                vt["pool"] += POOL_MASK
                                nc.scalar.activation(e_u8, s_in, COPY,
                                                     bias=C2, scale=C1)
                                treg = e_pair[:, i, ds(off, P)]
                                nc.gpsimd.tensor_mul(treg, treg, tri_f8)
                            else:
                                vt["dve"] += dve_c
                                nc.vector.scalar_tensor_tensor(
                                    e_u8, s_in, C1, maskbias[:, :w],
                                    MULT, ADD)
                    pv_queue.append((t, p, npair, offp, e_pair, o_ps,
                                     db_ps))
                    if len(pv_queue) > 3:
                        emit_pv(pv_queue.pop(0))

            for job in pv_queue:
                emit_pv(job)

    nc.compile()
    return nc


def _get_nc():
    if not _nc_cache:
        _nc_cache.append(_build())
    return _nc_cache[0]


def _prepare_inputs(query, key, value):
    B, H, W, C = query.shape
    n = H * W
    f8 = ml_dtypes.float8_e4m3
    bf = ml_dtypes.bfloat16
    q = np.asarray(query, np.float32).reshape(B, n, C)
    k = np.asarray(key, np.float32).reshape(B, n, C)
    v = np.asarray(value, np.float32).reshape(B, n, C)
    qT = np.ascontiguousarray(q.transpose(0, 2, 1))               # [B, C, n]
    kT = np.ascontiguousarray(k.transpose(0, 2, 1))
    qT8 = qT.astype(f8)
    kT8 = kT.astype(f8)
    qT0 = qT[:, :, :QCH].astype(bf)
    kT0 = kT[:, :, :QCH].astype(bf)
    # v8 [128, n]: v8[p, j*128 + c] = v[j*128 + p, c]
    v8 = np.ascontiguousarray(
        v.reshape(B, n // P, P, C).transpose(0, 2, 1, 3).reshape(B, P, n)
    ).astype(f8)
    v0 = np.ascontiguousarray(
        v[:, :QCH].reshape(B, 4, P, C).transpose(0, 2, 1, 3).reshape(B, P, 4 * C)
    ).astype(bf)
    return [
        {"qT8": qT8[b], "kT8": kT8[b], "qT0": qT0[b], "kT0": kT0[b],
         "v8": v8[b], "v0": v0[b]}
        for b in range(B)
    ]


def kernel(query, key, value):
    B, H, W, C = query.shape
    CV = value.shape[-1]
    n = H * W
    in_maps = _prepare_inputs(query, key, value)
    nc = _get_nc()
    res = run_bass_kernel_spmd(nc, in_maps, core_ids=list(range(N_CORES)))
    out = np.empty((B, n, CV), np.float32)
    for b in range(B):
        oT = res.results[b]["outT"].astype(np.float32)   # [128, n]
        dn = res.results[b]["den"]                       # [1, n]
        out[b] = (oT / dn).T
    return out.reshape(B, H, W, CV)
